# revision 1
# baseline (speedup 1.0000x reference)
"""IF spiking-neuron scan (charge / fire / hard-reset) on 8 Trainium2 cores.

Reference recurrence over t (elementwise on every [B, N] element):
    v = v + x_t
    s = (v - 1.0 >= 0)          # spike, 0.0/1.0
    v = (1 - s) * v             # hard reset to 0

Sharding: pure data parallel over the B*N = 262144 element dimension;
each of the 8 cores owns 32768 element chains [T=64, 32768] with zero
communication. Per core the elements live in SBUF as a [128, 256] f32
state tile; the 64-step scan runs locally. All arithmetic is fp32 and
bit-exact vs the reference (adds, compares, and mult-by-0/1 only).

Kernel structure:
  per timestep, two interleaved half-width streams (A/B) on the vector
  engine, so each op's producer is two instructions back and the DVE
  write-ack latency is hidden:
      u[A] = v[A] + x_t[A]        (tensor_tensor add)
      u[B] = v[B] + x_t[B]
      v[A] = (u[A] < 1) * u[A]    (scalar_tensor_tensor fused cmp+mult)
      v[B] = (u[B] < 1) * u[B]
  per block of timesteps, the otherwise-idle scalar engine derives the
  spikes from the batched u values with an exact two-op step function
  (exact even at u == V_TH):
      z = sign(V_TH - u)          in {-1, 0, 1}
      r = relu(z)                 in {0, 1},  r == 1 - s
  r travels to DRAM as uint8 in [partition, t, f] layout (2 KiB
  contiguous per partition per block -> line-rate DMA, 4x less output
  traffic); the host computes s = 1 - r in float32. Timestep blocks are
  small at the start/end of the scan to shrink the pipeline fill/drain.
"""

import numpy as np

import concourse.tile as tile
from concourse import bacc, mybir
from concourse.bass_utils import run_bass_kernel_spmd

T = 64
B = 32
N = 8192
NCORES = 8
PERCORE = (B * N) // NCORES  # 32768 element chains per core
P = 128                      # SBUF partitions
F = PERCORE // P             # 256 elements per partition
H = F // 2                   # half-width for the two interleaved streams

V_TH = 1.0

# timestep block sizes: small at the edges to cut pipeline fill/drain
BLOCKS = [2, 2, 4] + [8] * 6 + [4, 2, 2]
assert sum(BLOCKS) == T

_NC_CACHE = {}


def build_nc(blocks=None, spike_on_act=True):
    blocks = list(BLOCKS if blocks is None else blocks)
    # Bacc (not raw Bass): its compile() splits multi-wait sync conditions
    # into nop/event-semaphore prefixes — walrus accepts at most one sync
    # wait per hardware instruction.
    nc = bacc.Bacc("TRN2", target_bir_lowering=False, debug=False)
    x = nc.dram_tensor("x", [T, PERCORE], mybir.dt.float32, kind="ExternalInput").ap()
    y = nc.dram_tensor("y", [P, T, F], mybir.dt.uint8, kind="ExternalOutput").ap()

    # x: [T, P*F] -> [P, T, F]; per partition each timestep is a contiguous
    # 1 KiB run in DRAM. y is already [P, T, F]: per partition a block of
    # timesteps is one contiguous run.
    xr = x.rearrange("t (p f) -> p t f", p=P)

    with tile.TileContext(nc) as tc:
        with (
            tc.tile_pool(name="xin", bufs=4) as xpool,
            tc.tile_pool(name="sout", bufs=4) as spool,
            tc.tile_pool(name="ub", bufs=3) as ubpool,
            tc.tile_pool(name="zb", bufs=2) as zpool,
            tc.tile_pool(name="v", bufs=1) as vpool,
        ):
            v = vpool.tile([P, F], mybir.dt.float32)
            nc.vector.memset(v[:], 0.0)
            t0 = 0
            for tb in blocks:
                xt = xpool.tile([P, tb * F], mybir.dt.float32, tag="xin")
                if t0 == 0 and tb > 1:
                    # Block 0 gates the whole scan: split it per-timestep
                    # across both HW-DGE rings so the triggers run in
                    # parallel and the first timestep lands sooner.
                    nc.sync.dma_start(xt[:, :F], xr[:, 0:1, :])
                    nc.scalar.dma_start(xt[:, F:], xr[:, 1:tb, :])
                else:
                    nc.sync.dma_start(xt[:], xr[:, t0:t0 + tb, :])
                ub = ubpool.tile([P, tb * F], mybir.dt.float32, tag="ub")
                for ti in range(tb):
                    for h in range(2):
                        lo = ti * F + h * H
                        nc.vector.tensor_add(
                            ub[:, lo:lo + H], v[:, h * H:(h + 1) * H],
                            xt[:, lo:lo + H],
                        )
                    if t0 + ti == T - 1:
                        continue  # v after the final timestep is never read
                    for h in range(2):
                        lo = ti * F + h * H
                        nc.vector.scalar_tensor_tensor(
                            v[:, h * H:(h + 1) * H], ub[:, lo:lo + H], V_TH,
                            ub[:, lo:lo + H],
                            mybir.AluOpType.is_lt, mybir.AluOpType.mult,
                        )
                st = spool.tile([P, tb * F], mybir.dt.uint8, tag="sout")
                # Final blocks: the DVE is idle once its scan ends, while the
                # scalar engine still owes Sign+Relu for the last u values —
                # a pure tail. Computing those spikes on the DVE as
                # r = (u < 1) removes the ACT tail before the last DMA.
                last_two = t0 + tb > T - 5
                if spike_on_act and not last_two:
                    # Spike path on the otherwise-idle scalar engine, exact
                    # even when u == V_TH:  z = sign(V_TH - u) in {-1,0,1},
                    # r = relu(z) in {0,1}; r == 1 - s, host flips it back.
                    zt = zpool.tile([P, tb * F], mybir.dt.float32, tag="zb")
                    nc.scalar.activation(
                        zt[:], ub[:], mybir.ActivationFunctionType.Sign,
                        bias=V_TH, scale=-1.0,
                    )
                    nc.scalar.activation(
                        st[:], zt[:], mybir.ActivationFunctionType.Relu,
                    )
                else:
                    # r = (u < V_TH) == 1 - s, same polarity as the ACT path
                    nc.vector.tensor_scalar(
                        st[:], ub[:], V_TH, None, mybir.AluOpType.is_lt
                    )
                # outputs ride the scalar engine's HW-DGE ring so input
                # triggers never queue behind them on the SP ring
                nc.scalar.dma_start(y[:, t0:t0 + tb, :], st[:])
                t0 += tb
    nc.compile()
    return nc


def _get_nc():
    if "nc" not in _NC_CACHE:
        _NC_CACHE["nc"] = build_nc()
    return _NC_CACHE["nc"]


def run_sharded(x_seq, trace=False, nc=None, spike_on_act=True, **kwargs):
    if nc is None:
        nc = _get_nc()
    x2 = np.ascontiguousarray(np.asarray(x_seq, dtype=np.float32)).reshape(T, B * N)
    in_maps = [
        {"x": np.ascontiguousarray(x2[:, c * PERCORE:(c + 1) * PERCORE])}
        for c in range(NCORES)
    ]
    # A cold device occasionally reports NRT_EXEC_UNIT_UNRECOVERABLE on the
    # first execute and recovers on the next attempt; retry a couple times.
    for attempt in range(3):
        try:
            res = run_bass_kernel_spmd(
                nc, in_maps, list(range(NCORES)), trace=trace, **kwargs
            )
            break
        except Exception:  # jax.errors.JaxRuntimeError and friends
            if attempt == 2:
                raise
            import time
            time.sleep(2.0)
    out = np.empty((T, B * N), dtype=np.float32)
    for c in range(NCORES):
        yc = np.asarray(res.results[c]["y"])          # [P, T, F] uint8
        r = yc.transpose(1, 0, 2).reshape(T, PERCORE)
        # device stores r = 1 - s on every path
        out[:, c * PERCORE:(c + 1) * PERCORE] = 1 - r
    return out.reshape(T, B, N), res


def kernel(x_seq):
    out, _ = run_sharded(x_seq)
    return out



# revision 2
# speedup vs baseline: 1.3164x; 1.3164x over previous
"""IF spiking-neuron scan (charge / fire / hard-reset) on 8 Trainium2 cores.

Reference recurrence over t (elementwise on every [B, N] element):
    u_t = v_{t-1} + x_t          # charge
    s_t = (u_t >= 1.0)           # fire
    v_t = (1 - s_t) * u_t        # hard reset to 0

Sharding: pure data parallel over the B*N = 262144 element chains; each
of the 8 cores owns 32768 chains laid out as a [128, 256] tile per
timestep, with zero communication.

Kernel structure (one custom DVE op per timestep, spikes on ACT):

  The recurrence is rewritten on the u-sequence:
      u_{t+1} = (u_t if u_t < 1 else 0) + x_{t+1}
  which is ONE fused DVE instruction per timestep via a custom DVE op
  (registered at import):
      IF_STEP_ANT: out = select(Src0 < C0, Src0, Zero) + Src1
  This halves the vector-engine work vs the classic add + cmp/mult pair
  and keeps the pre-reset potential u_t materialized in SBUF, so the
  spike extraction runs on the otherwise-idle scalar engine off the
  critical path:
      r_t = Sign(1.0 - u_t)  ->  uint8
  r == 1 exactly when u < 1 (no spike); u >= 1 gives 0 or 255 (0.0 or
  -1.0 cast to uint8, saturate or wrap - both decode the same), so the
  host computes s = (r != 1). Sign(0) = 0 keeps u == V_TH exact.

  All fp32 arithmetic (the single add and the compare) is bit-identical
  to the reference. Input streams [P, t, F] on the sync-ring DMA queues,
  spike bytes return on the scalar-ring queues; with DVE at ~21 us and
  ACT at ~16 us the kernel is bound by the ~29 us of HBM traffic
  (8.4 MB in + 2.1 MB out per core at ~360 GB/s).
"""

import numpy as np

import concourse.tile as tile
from concourse import bacc, mybir
from concourse.bass_utils import run_bass_kernel_spmd

T = 64
B = 32
N = 8192
NCORES = 8
PERCORE = (B * N) // NCORES  # 32768 element chains per core
P = 128                      # SBUF partitions
F = PERCORE // P             # 256 elements per partition

V_TH = 1.0

# timestep block sizes: small at the edges to cut pipeline fill/drain
BLOCKS = [1, 1, 2, 4] + [8] * 6 + [4, 2, 1, 1]
assert sum(BLOCKS) == T

_NC_CACHE = {}
_OP_CACHE = {}


def _register_if_step_op():
    """Register the fused IF-neuron step as a custom DVE op.

    Uses the documented extension point (concourse.dve_ops.OPS): the op
    body lowers to a single steady-state uop program whose sha is pinned
    at registration, the sub-opcode row is taken from the free range
    [1, 0x20), and the numpy reference makes CoreSim scheduling exact.
    """
    if "op" in _OP_CACHE:
        return _OP_CACHE["op"]

    import concourse.dve_ops as dve_ops
    from concourse.dve_spec import Spec, Src0, Src1, C0, Zero, select, lower, _has_src1
    from concourse.dve_uop import DveOpSpec

    name = "IF_STEP_ANT"

    def _ref(in0, in1, c0, c1, c2):
        u = np.where(
            in0.astype(np.float32) < np.float32(c0),
            in0.astype(np.float32),
            np.float32(0.0),
        ).astype(np.float32)
        return (u + in1.astype(np.float32)).astype(np.float32)

    spec = Spec(body=select(Src0 < C0, Src0, Zero) + Src1, reference=_ref)

    existing = {op.name: op for op in dve_ops.OPS}
    if name in existing:
        _OP_CACHE["op"] = existing[name]
        return existing[name]

    row = 1 + len(dve_ops.OPS)
    shas = {}
    for ver in ("v3", "v4"):
        try:
            uops = lower(spec, ver=ver)
            shas[ver] = DveOpSpec(
                name=name, opcode=row, uops=uops, rd1_en=_has_src1(spec)
            ).sha(ver)
        except Exception:
            pass  # ver not supported in this build; TRN2 only needs v3

    op = dve_ops.DveOp(name, spec, subdim=False, uops_sha=shas)
    dve_ops.OPS.append(op)
    dve_ops._SUB_OPCODE_FOR_NAME[name] = row
    dve_ops.CUSTOM_DVE_SPECS[name] = spec
    _OP_CACHE["op"] = op
    return op


def build_nc(blocks=None):
    blocks = list(BLOCKS if blocks is None else blocks)
    if_step = _register_if_step_op()
    # Bacc (not raw Bass): its compile() splits multi-wait sync conditions
    # into nop/event-semaphore prefixes — walrus accepts at most one sync
    # wait per hardware instruction.
    nc = bacc.Bacc("TRN2", target_bir_lowering=False, debug=False)
    x = nc.dram_tensor("x", [T, PERCORE], mybir.dt.float32, kind="ExternalInput").ap()
    y = nc.dram_tensor("y", [P, T, F], mybir.dt.uint8, kind="ExternalOutput").ap()

    # x: [T, P*F] -> [P, T, F]; per partition each timestep is a contiguous
    # 1 KiB run in DRAM. y is already [P, T, F]: per partition a block of
    # timesteps is one contiguous run.
    xr = x.rearrange("t (p f) -> p t f", p=P)

    with tile.TileContext(nc) as tc:
        with (
            tc.tile_pool(name="xin", bufs=4) as xpool,
            tc.tile_pool(name="ub", bufs=4) as upool,
            tc.tile_pool(name="sout", bufs=4) as spool,
            tc.tile_pool(name="z", bufs=1) as zpool,
        ):
            zero = zpool.tile([P, F], mybir.dt.float32)
            nc.vector.memset(zero[:], 0.0)
            prev = zero  # tile holding u_{t-1} in its last F-slice
            prev_lo = 0
            t0 = 0
            for tb in blocks:
                xt = xpool.tile([P, tb * F], mybir.dt.float32, tag="xin")
                nc.sync.dma_start(xt[:], xr[:, t0:t0 + tb, :])
                ub = upool.tile([P, tb * F], mybir.dt.float32, tag="ub")
                for ti in range(tb):
                    lo = ti * F
                    nc.vector._custom_dve(
                        if_step,
                        out=ub[:, lo:lo + F],
                        in0=prev[:, prev_lo:prev_lo + F],
                        in1=xt[:, lo:lo + F],
                        s0=V_TH,
                    )
                    prev, prev_lo = ub, lo
                st = spool.tile([P, tb * F], mybir.dt.uint8, tag="sout")
                # r = Sign(V_TH - u) cast to uint8: 1 <=> no spike; spike
                # rows are 0 (u == V_TH) or the cast of -1.0 (saturate 0 /
                # wrap 255). Host decodes s = (r != 1); Sign(0) = 0 keeps
                # exact threshold ties correct.
                nc.scalar.activation(
                    st[:], ub[:], mybir.ActivationFunctionType.Sign,
                    bias=V_TH, scale=-1.0,
                )
                # outputs ride the scalar engine's HW-DGE ring so input
                # triggers never queue behind them on the SP ring
                nc.scalar.dma_start(y[:, t0:t0 + tb, :], st[:])
                t0 += tb
    nc.compile()
    return nc


def _get_nc():
    if "nc" not in _NC_CACHE:
        _NC_CACHE["nc"] = build_nc()
    return _NC_CACHE["nc"]


def run_sharded(x_seq, trace=False, nc=None, **kwargs):
    if nc is None:
        nc = _get_nc()
    x2 = np.ascontiguousarray(np.asarray(x_seq, dtype=np.float32)).reshape(T, B * N)
    in_maps = [
        {"x": np.ascontiguousarray(x2[:, c * PERCORE:(c + 1) * PERCORE])}
        for c in range(NCORES)
    ]
    # A cold device occasionally reports NRT_EXEC_UNIT_UNRECOVERABLE on the
    # first execute and recovers on the next attempt; retry a couple times.
    for attempt in range(3):
        try:
            res = run_bass_kernel_spmd(
                nc, in_maps, list(range(NCORES)), trace=trace, **kwargs
            )
            break
        except Exception:  # jax.errors.JaxRuntimeError and friends
            if attempt == 2:
                raise
            import time
            time.sleep(2.0)
    out = np.empty((T, B * N), dtype=np.float32)
    for c in range(NCORES):
        yc = np.asarray(res.results[c]["y"])          # [P, T, F] uint8
        r = yc.transpose(1, 0, 2).reshape(T, PERCORE)
        # r == 1 <=> no spike (u < V_TH); 0 and 255 both mean spike
        out[:, c * PERCORE:(c + 1) * PERCORE] = (r != 1)
    return out.reshape(T, B, N), res


def kernel(x_seq):
    out, _ = run_sharded(x_seq)
    return out
